# revision 34
# baseline (speedup 1.0000x reference)
"""BinaryLinear kernel for Trainium2 (8 NeuronCores, SPMD). v9.

y = x @ sign(W)^T + sign(b); x[8192,4096] W[4096,4096] b[4096] f32.
Sharding: tokens 2-way x out_features 4-way -> per core
x[4096,4096] W[1024,4096] b[1024] -> y[4096,1024].

Single bf16 pass (~1.2e-3 max-metric rel err vs 2e-2 tolerance).

v9: W^T is built on the (otherwise idle) TensorEngine during phase 0
via is_transpose matmuls against an identity, 128x128 per shot, with
PSUM->swt evictions alternating DVE/ACT. This removes the eight 1MB
xbar W transposes (~42us of serial SDMA time) from phase 0; the xbar
ring then serves x tiles 0-3 during phase 0, so the steady pipeline
starts primed. y stores ride the ACT HWDGE ring (v8: keeps the
gpsimd ring loads decoupled from PE progress).

Known hardware behavior baked in:
  - Copies and xbar transposes are strictly additive on the 16 SDMA
    engines; phase-0 time ~= loads + transposes unless transposes
    move off the SDMA path entirely (this version).
  - DMA union busy was the 524-553us invariant across v1-v8 at
    ~143MB moved; this drops it to ~500us.
  - Run-to-run clock state (2.0 vs 2.4 GHz PE) swings totals ~6%.
"""

import sys

sys.path.insert(0, "/opt/trn_rl_repo")

import numpy as np

import concourse.bass as bass  # noqa: F401
import concourse.mybir as mybir
from concourse import bacc, tile
from concourse.bass_utils import run_bass_kernel_spmd
from concourse.masks import make_identity

TOKENS, IN, OUT = 8192, 4096, 4096
N_CORES = 8
T_SPLIT, O_SPLIT = 2, 4
T_CORE, O_CORE = TOKENS // T_SPLIT, OUT // O_SPLIT

P = 128
FREE = 512

F32 = mybir.dt.float32
BF16 = mybir.dt.bfloat16
U16 = mybir.dt.uint16


def emit(nc, tc, x_d, w_d, b_d, y_d, t_core, in_dim, o_core):
    KS = in_dim // P
    TT = t_core // P
    OT = o_core // P

    from contextlib import ExitStack

    with ExitStack() as ctx:
        const = ctx.enter_context(tc.tile_pool(name="const", bufs=1))
        swt = const.tile([P, KS, o_core], BF16)
        bias_bc = const.tile([P, o_core], F32)
        ident = const.tile([P, P], BF16)
        make_identity(nc, ident)

        xpool = ctx.enter_context(tc.tile_pool(name="xload", bufs=2))
        hpool = ctx.enter_context(tc.tile_pool(name="hilo", bufs=1))
        tpool = ctx.enter_context(tc.tile_pool(name="xt", bufs=4))
        psum = ctx.enter_context(tc.tile_pool(name="psum", bufs=6, space="PSUM"))
        opool = ctx.enter_context(tc.tile_pool(name="yout", bufs=3))

        def prep_tile(tt):
            """x f32 load -> bf16 cast (DVE) -> xbar transpose."""
            trow = slice(tt * P, (tt + 1) * P)
            xf = xpool.tile([P, in_dim], F32, name="xf")
            nc.gpsimd.dma_start(xf, x_d[trow, :])
            xhi = hpool.tile([P, in_dim], BF16, name="xhi")
            nc.vector.tensor_copy(out=xhi, in_=xf)
            xhiT = tpool.tile([P, KS, P], BF16, name="xhiT")
            nc.sync.dma_start_transpose(xhiT, xhi)
            return xhiT

        def sweep_og(tt, xhiT, og, yo):
            ocol = slice(og * FREE, (og + 1) * FREE)
            ps = psum.tile([P, FREE], F32, name="ps", bufs=4)
            for ks in range(KS):
                nc.tensor.matmul(
                    ps, xhiT[:, ks, :], swt[:, ks, ocol],
                    start=(ks == 0), stop=(ks == KS - 1),
                )
            nc.vector.tensor_tensor(
                out=yo[:, ocol], in0=ps,
                in1=bias_bc[:, ocol], op=mybir.AluOpType.add,
            )

        with tc.tile_pool(name="wload", bufs=1) as wpool:
            braw = wpool.tile([P, o_core], F32, name="braw", bufs=1)
            nc.gpsimd.dma_start(braw, b_d.to_broadcast([P, o_core]))
            nc.scalar.sign(bias_bc, braw)

            # x tiles 0-3 prepped during phase 0: the xbar ring has no
            # W transposes to fight, only the W/x load copies (drains)
            pend = {}
            for ot in range(OT):
                wf = wpool.tile([P, in_dim], F32, name="wf", bufs=2)
                nc.scalar.dma_start(wf, w_d[ot * P : (ot + 1) * P, :])
                ws = wpool.tile([P, in_dim], BF16, name="ws", bufs=2)
                nc.vector.tensor_scalar(
                    out=ws.bitcast(U16),
                    in0=wf.bitcast(U16)[:, 1::2],
                    scalar1=0x8000,
                    scalar2=0x3F80,
                    op0=mybir.AluOpType.bitwise_and,
                    op1=mybir.AluOpType.bitwise_or,
                )
                if ot < 4:
                    pend[ot] = prep_tile(ot)
                # PE transposes, 8 slabs batched into one full PSUM
                # bank (one eviction per bank amortizes the ~2us
                # cross-engine semaphore round trip), evictions
                # alternating DVE/ACT
                for kg in range(KS // 16):
                    psT = psum.tile([P, 16, P], BF16, name="psT", bufs=2)
                    for j in range(16):
                        ks = kg * 16 + j
                        nc.tensor.transpose(
                            psT[:, j, :], ws[:, ks * P : (ks + 1) * P],
                            ident,
                        )
                    dst = swt[:, kg * 16 : (kg + 1) * 16, ot * P : (ot + 1) * P]
                    if kg % 2 == 0:
                        nc.vector.tensor_copy(out=dst, in_=psT)
                    else:
                        nc.scalar.copy(dst, psT)
        # ---- Phase 1 ----
        def mm_tile(tt, xhiT):
            yo = opool.tile([P, o_core], F32, name="yo")
            for og in range(o_core // FREE):
                sweep_og(tt, xhiT, og, yo)
            nc.scalar.dma_start(y_d[tt * P : (tt + 1) * P, :], yo)

        for tt in range(TT):
            mm_tile(tt, pend.pop(tt))
            if tt + 4 < TT:
                pend[tt + 4] = prep_tile(tt + 4)


def build(t_core=T_CORE, in_dim=IN, o_core=O_CORE):
    nc = bacc.Bacc("TRN2", target_bir_lowering=False, debug=False)
    x_d = nc.dram_tensor("x", [t_core, in_dim], F32, kind="ExternalInput")
    w_d = nc.dram_tensor("w", [o_core, in_dim], F32, kind="ExternalInput")
    b_d = nc.dram_tensor("b", [1, o_core], F32, kind="ExternalInput")
    y_d = nc.dram_tensor("y", [t_core, o_core], F32, kind="ExternalOutput")
    with tile.TileContext(nc) as tc:
        emit(nc, tc, x_d.ap(), w_d.ap(), b_d.ap(), y_d.ap(), t_core, in_dim, o_core)
    nc.compile()
    return nc


_nc_cache = None


def kernel(x: np.ndarray, weight: np.ndarray, bias: np.ndarray, **run_kwargs):
    global _nc_cache
    if _nc_cache is None:
        _nc_cache = build()
    nc = _nc_cache

    x = np.ascontiguousarray(x, dtype=np.float32)
    weight = np.ascontiguousarray(weight, dtype=np.float32)
    bias = np.ascontiguousarray(bias, dtype=np.float32)

    in_maps = []
    for c in range(N_CORES):
        th, oq = divmod(c, O_SPLIT)
        in_maps.append(
            {
                "x": x[th * T_CORE : (th + 1) * T_CORE],
                "w": weight[oq * O_CORE : (oq + 1) * O_CORE],
                "b": bias[oq * O_CORE : (oq + 1) * O_CORE].reshape(1, O_CORE),
            }
        )
    res = run_bass_kernel_spmd(nc, in_maps, core_ids=list(range(N_CORES)), **run_kwargs)
    y = np.empty((TOKENS, OUT), dtype=np.float32)
    for c in range(N_CORES):
        th, oq = divmod(c, O_SPLIT)
        y[th * T_CORE : (th + 1) * T_CORE, oq * O_CORE : (oq + 1) * O_CORE] = (
            res.results[c]["y"]
        )
    kernel.last_results = res
    return y


# revision 36
# speedup vs baseline: 1.0483x; 1.0483x over previous
"""BinaryLinear kernel for Trainium2 (8 NeuronCores, SPMD). v9.

y = x @ sign(W)^T + sign(b); x[8192,4096] W[4096,4096] b[4096] f32.
Sharding: tokens 2-way x out_features 4-way -> per core
x[4096,4096] W[1024,4096] b[1024] -> y[4096,1024].

Single bf16 pass (~1.2e-3 max-metric rel err vs 2e-2 tolerance).

v9: W^T is built on the (otherwise idle) TensorEngine during phase 0
via is_transpose matmuls against an identity, 128x128 per shot, with
PSUM->swt evictions alternating DVE/ACT. This removes the eight 1MB
xbar W transposes (~42us of serial SDMA time) from phase 0; the xbar
ring then serves x tiles 0-3 during phase 0, so the steady pipeline
starts primed. y stores ride the ACT HWDGE ring (v8: keeps the
gpsimd ring loads decoupled from PE progress).

Known hardware behavior baked in:
  - Copies and xbar transposes are strictly additive on the 16 SDMA
    engines; phase-0 time ~= loads + transposes unless transposes
    move off the SDMA path entirely (this version).
  - DMA union busy was the 524-553us invariant across v1-v8 at
    ~143MB moved; this drops it to ~500us.
  - Run-to-run clock state (2.0 vs 2.4 GHz PE) swings totals ~6%.
"""

import sys

sys.path.insert(0, "/opt/trn_rl_repo")

import numpy as np

import concourse.bass as bass  # noqa: F401
import concourse.mybir as mybir
from concourse import bacc, tile
from concourse.bass_utils import run_bass_kernel_spmd
from concourse.masks import make_identity

TOKENS, IN, OUT = 8192, 4096, 4096
N_CORES = 8
T_SPLIT, O_SPLIT = 2, 4
T_CORE, O_CORE = TOKENS // T_SPLIT, OUT // O_SPLIT

P = 128
FREE = 512

F32 = mybir.dt.float32
BF16 = mybir.dt.bfloat16
U16 = mybir.dt.uint16


def emit(nc, tc, x_d, w_d, b_d, y_d, t_core, in_dim, o_core):
    KS = in_dim // P
    TT = t_core // P
    OT = o_core // P

    from contextlib import ExitStack

    with ExitStack() as ctx:
        const = ctx.enter_context(tc.tile_pool(name="const", bufs=1))
        swt = const.tile([P, KS, o_core], BF16)
        bias_bc = const.tile([P, o_core], F32)
        ident = const.tile([P, P], BF16)
        make_identity(nc, ident)

        xpool = ctx.enter_context(tc.tile_pool(name="xload", bufs=2))
        hpool = ctx.enter_context(tc.tile_pool(name="hilo", bufs=1))
        tpool = ctx.enter_context(tc.tile_pool(name="xt", bufs=4))
        psum = ctx.enter_context(tc.tile_pool(name="psum", bufs=6, space="PSUM"))
        opool = ctx.enter_context(tc.tile_pool(name="yout", bufs=3))

        def prep_tile(tt):
            """x f32 load -> bf16 cast (DVE) -> xbar transpose."""
            trow = slice(tt * P, (tt + 1) * P)
            xf = xpool.tile([P, in_dim], F32, name="xf")
            nc.gpsimd.dma_start(xf, x_d[trow, :])
            xhi = hpool.tile([P, in_dim], BF16, name="xhi")
            nc.vector.tensor_copy(out=xhi, in_=xf)
            xhiT = tpool.tile([P, KS, P], BF16, name="xhiT")
            nc.sync.dma_start_transpose(xhiT, xhi)
            return xhiT

        def sweep_og(tt, xhiT, og, yo):
            ocol = slice(og * FREE, (og + 1) * FREE)
            ps = psum.tile([P, FREE], F32, name="ps", bufs=6)
            for ks in range(KS):
                nc.tensor.matmul(
                    ps, xhiT[:, ks, :], swt[:, ks, ocol],
                    start=(ks == 0), stop=(ks == KS - 1),
                )
            nc.vector.tensor_tensor(
                out=yo[:, ocol], in0=ps,
                in1=bias_bc[:, ocol], op=mybir.AluOpType.add,
            )

        with tc.tile_pool(name="wload", bufs=1) as wpool:
            braw = wpool.tile([P, o_core], F32, name="braw", bufs=1)
            nc.gpsimd.dma_start(braw, b_d.to_broadcast([P, o_core]))
            nc.scalar.sign(bias_bc, braw)

            # x tiles 0-3 prepped during phase 0: the xbar ring has no
            # W transposes to fight, only the W/x load copies (drains)
            pend = {}
            for ot in range(OT):
                wf = wpool.tile([P, in_dim], F32, name="wf", bufs=2)
                nc.scalar.dma_start(wf, w_d[ot * P : (ot + 1) * P, :])
                ws = wpool.tile([P, in_dim], BF16, name="ws", bufs=2)
                nc.vector.tensor_scalar(
                    out=ws.bitcast(U16),
                    in0=wf.bitcast(U16)[:, 1::2],
                    scalar1=0x8000,
                    scalar2=0x3F80,
                    op0=mybir.AluOpType.bitwise_and,
                    op1=mybir.AluOpType.bitwise_or,
                )
                if ot < 4:
                    pend[ot] = prep_tile(ot)
                if ot >= 6:
                    # last two W tiles go back on the xbar ring (it
                    # has headroom in phase 0) to shorten the PE
                    # transpose chain by ~25%
                    nc.sync.dma_start_transpose(
                        swt[:, :, ot * P : (ot + 1) * P], ws
                    )
                    continue
                # PE transposes, 8 slabs batched into one full PSUM
                # bank (one eviction per bank amortizes the ~2us
                # cross-engine semaphore round trip), evictions
                # alternating DVE/ACT
                for kg in range(KS // 8):
                    psT = psum.tile([P, 8, P], BF16, name="psT", bufs=2)
                    for j in range(8):
                        ks = kg * 8 + j
                        nc.tensor.transpose(
                            psT[:, j, :], ws[:, ks * P : (ks + 1) * P],
                            ident,
                        )
                    dst = swt[:, kg * 8 : (kg + 1) * 8, ot * P : (ot + 1) * P]
                    if kg % 2 == 0:
                        nc.vector.tensor_copy(out=dst, in_=psT)
                    else:
                        nc.scalar.copy(dst, psT)
        # ---- Phase 1 ----
        def mm_tile(tt, xhiT):
            yo = opool.tile([P, o_core], F32, name="yo")
            for og in range(o_core // FREE):
                sweep_og(tt, xhiT, og, yo)
            nc.scalar.dma_start(y_d[tt * P : (tt + 1) * P, :], yo)

        for tt in range(TT):
            mm_tile(tt, pend.pop(tt))
            if tt + 4 < TT:
                pend[tt + 4] = prep_tile(tt + 4)


def build(t_core=T_CORE, in_dim=IN, o_core=O_CORE):
    nc = bacc.Bacc("TRN2", target_bir_lowering=False, debug=False)
    x_d = nc.dram_tensor("x", [t_core, in_dim], F32, kind="ExternalInput")
    w_d = nc.dram_tensor("w", [o_core, in_dim], F32, kind="ExternalInput")
    b_d = nc.dram_tensor("b", [1, o_core], F32, kind="ExternalInput")
    y_d = nc.dram_tensor("y", [t_core, o_core], F32, kind="ExternalOutput")
    with tile.TileContext(nc) as tc:
        emit(nc, tc, x_d.ap(), w_d.ap(), b_d.ap(), y_d.ap(), t_core, in_dim, o_core)
    nc.compile()
    return nc


_nc_cache = None


def kernel(x: np.ndarray, weight: np.ndarray, bias: np.ndarray, **run_kwargs):
    global _nc_cache
    if _nc_cache is None:
        _nc_cache = build()
    nc = _nc_cache

    x = np.ascontiguousarray(x, dtype=np.float32)
    weight = np.ascontiguousarray(weight, dtype=np.float32)
    bias = np.ascontiguousarray(bias, dtype=np.float32)

    in_maps = []
    for c in range(N_CORES):
        th, oq = divmod(c, O_SPLIT)
        in_maps.append(
            {
                "x": x[th * T_CORE : (th + 1) * T_CORE],
                "w": weight[oq * O_CORE : (oq + 1) * O_CORE],
                "b": bias[oq * O_CORE : (oq + 1) * O_CORE].reshape(1, O_CORE),
            }
        )
    res = run_bass_kernel_spmd(nc, in_maps, core_ids=list(range(N_CORES)), **run_kwargs)
    y = np.empty((TOKENS, OUT), dtype=np.float32)
    for c in range(N_CORES):
        th, oq = divmod(c, O_SPLIT)
        y[th * T_CORE : (th + 1) * T_CORE, oq * O_CORE : (oq + 1) * O_CORE] = (
            res.results[c]["y"]
        )
    kernel.last_results = res
    return y
